# revision 12
# baseline (speedup 1.0000x reference)
import numpy as np
import ml_dtypes
import concourse.bass as bass
import concourse.bacc as bacc
import concourse.tile as tile
from concourse import mybir
from concourse.bass_utils import run_bass_kernel_spmd

BF16 = ml_dtypes.bfloat16
f32, bf = mybir.dt.float32, mybir.dt.bfloat16
AF = mybir.ActivationFunctionType
ALU = mybir.AluOpType

N_CORES = 8
B, SQ, SKV, D, H, FFW, HD = 4, 512, 4096, 1024, 16, 4096, 64
EPS = 1e-5

_cache = {}


def _ln(nc, tmp, epst, x, z_out):
    st = tmp.tile([128, 12], f32, tag="ln_st", name="ln_st")
    nc.vector.bn_stats(out=st[:, 0:6], in_=x[:, 0:512])
    nc.vector.bn_stats(out=st[:, 6:12], in_=x[:, 512:1024])
    mv = tmp.tile([128, 2], f32, tag="ln_mv", name="ln_mv")
    nc.vector.bn_aggr(out=mv, in_=st)
    sd = tmp.tile([128, 1], f32, tag="ln_sd", name="ln_sd")
    nc.scalar.activation(out=sd, in_=mv[:, 1:2], func=AF.Sqrt, bias=epst)
    rs = tmp.tile([128, 1], f32, tag="ln_rs", name="ln_rs")
    nc.vector.reciprocal(out=rs, in_=sd)
    nc.vector.tensor_scalar(
        out=z_out, in0=x, scalar1=mv[:, 0:1], scalar2=rs,
        op0=ALU.subtract, op1=ALU.mult,
    )


def _build():
    if "nc" in _cache:
        return _cache["nc"]
    nc = bacc.Bacc("TRN2", target_bir_lowering=False, debug=False,
                   num_devices=N_CORES)
    qs_d = nc.dram_tensor("qs", [256, D], f32, kind="ExternalInput").ap()
    kvs_d = nc.dram_tensor("kvs", [2048, D], f32, kind="ExternalInput").ap()
    mask_d = nc.dram_tensor("maskb", [B, SKV], bf, kind="ExternalInput").ap()
    wq_d = nc.dram_tensor("wq", [D, D], bf, kind="ExternalInput").ap()
    wk_d = nc.dram_tensor("wk", [D, D], bf, kind="ExternalInput").ap()
    wv_d = nc.dram_tensor("wv", [D, D], bf, kind="ExternalInput").ap()
    wo_d = nc.dram_tensor("wo", [D, D], bf, kind="ExternalInput").ap()
    w1_d = nc.dram_tensor("w1", [D, FFW], bf, kind="ExternalInput").ap()
    w2_d = nc.dram_tensor("w2", [FFW, D], bf, kind="ExternalInput").ap()
    bq_d = nc.dram_tensor("bq2", [128, 8], f32, kind="ExternalInput").ap()
    bk_d = nc.dram_tensor("bk2", [128, 8], f32, kind="ExternalInput").ap()
    b1_d = nc.dram_tensor("b12", [128, 32], f32, kind="ExternalInput").ap()
    bvr_d = nc.dram_tensor("bvr", [1, D], f32, kind="ExternalInput").ap()
    bor_d = nc.dram_tensor("bor", [1, D], f32, kind="ExternalInput").ap()
    b2r_d = nc.dram_tensor("b2r", [1, D], f32, kind="ExternalInput").ap()
    id_d = nc.dram_tensor("ident", [128, 128], bf, kind="ExternalInput").ap()
    eps_d = nc.dram_tensor("epsc", [128, 1], f32, kind="ExternalInput").ap()
    out_d = nc.dram_tensor("out", [256, D], f32, kind="ExternalOutput").ap()
    attn_d = nc.dram_tensor("attn", [B * 2 * SQ, SKV], f32,
                            kind="ExternalOutput").ap()

    with tile.TileContext(nc) as tc:
        with tc.tile_pool(name="pp", bufs=1) as pp, \
             tc.tile_pool(name="pss", bufs=4, space="PSUM") as pss, \
             tc.tile_pool(name="pst", bufs=2, space="PSUM") as pst, \
             tc.tile_pool(name="psa", bufs=2, space="PSUM") as psa, \
             tc.tile_pool(name="vdp", bufs=1, space="DRAM") as vdp, \
             tc.tile_pool(name="rsp", bufs=2, space="DRAM") as rsp:
            identt = pp.tile([128, 128], bf, tag="id", name="id")
            nc.sync.dma_start(out=identt, in_=id_d)
            epst = pp.tile([128, 1], f32, tag="eps", name="eps")
            nc.sync.dma_start(out=epst, in_=eps_d)
            qs_t = []
            for rt in range(2):
                t = pp.tile([128, D], f32, tag=f"qs{rt}", name=f"qs{rt}")
                nc.sync.dma_start(out=t, in_=qs_d[rt * 128:(rt + 1) * 128, :])
                qs_t.append(t)
            aoT = [pp.tile([128, 256], bf, tag=f"ao{dc}", name=f"ao{dc}") for dc in range(8)]
            oat2 = [pp.tile([128, D], f32, tag=f"oa2{rt}", name=f"oa2{rt}") for rt in range(2)]
            z2T = [pp.tile([128, 256], bf, tag=f"z2t{dc}", name=f"z2t{dc}") for dc in range(8)]

            with tc.tile_pool(name="apool", bufs=1) as apool, \
                 tc.tile_pool(name="tmp", bufs=3) as tmp, \
                 tc.tile_pool(name="pab", bufs=3) as pab, \
                 tc.tile_pool(name="vhp", bufs=2) as vhp:
                wq_s, wk_s, wv_s = [], [], []
                for lst, src, nm, eng in ((wq_s, wq_d, "wq", nc.sync),
                                          (wk_s, wk_d, "wk", nc.scalar),
                                          (wv_s, wv_d, "wv", nc.scalar)):
                    for dc in range(8):
                        t = apool.tile([128, D], bf, tag=f"{nm}{dc}", name=f"{nm}{dc}")
                        eng.dma_start(
                            out=t, in_=src[dc * 128:(dc + 1) * 128, :])
                        lst.append(t)
                bq_sb = apool.tile([128, 8], f32, tag="bq", name="bq")
                nc.sync.dma_start(out=bq_sb, in_=bq_d)
                bk_sb = apool.tile([128, 8], f32, tag="bk", name="bk")
                nc.sync.dma_start(out=bk_sb, in_=bk_d)
                bv_b = apool.tile([128, D], f32, tag="bvb", name="bvb")
                nc.gpsimd.dma_start(out=bv_b, in_=bass.AP(
                    tensor=bvr_d.tensor, offset=bvr_d.offset,
                    ap=[[0, 128], [1, D]]))

                # ---- Phase Q: LN(q), transpose, Q-projection, head rearrange
                zq = []
                for rt in range(2):
                    t = apool.tile([128, D], bf, tag=f"zq{rt}", name=f"zq{rt}")
                    _ln(nc, tmp, epst, qs_t[rt], t)
                    zq.append(t)
                zqT = [apool.tile([128, 256], bf, tag=f"zqT{dc}", name=f"zqT{dc}")
                       for dc in range(8)]
                for dc in range(8):
                    for rt in range(2):
                        pt = pst.tile([128, 128], bf, tag="tr", name="tr")
                        nc.tensor.transpose(
                            pt, zq[rt][:, dc * 128:(dc + 1) * 128], identt)
                        nc.vector.tensor_copy(
                            out=zqT[dc][:, rt * 128:(rt + 1) * 128], in_=pt)
                qh = {}
                for bb in range(B):
                    for hl in range(2):
                        t = apool.tile([65, 512], bf, tag=f"qh{bb}{hl}", name=f"qh{bb}{hl}")
                        nc.vector.memset(t[64:65, :], 1.0)
                        qh[bb, hl] = t
                for oc in range(8):
                    qpt = pss.tile([128, 512], f32, tag="s", name="s")
                    qp = qpt[:, 0:256]
                    for dc in range(8):
                        nc.tensor.matmul(
                            qp, wq_s[dc][:, oc * 128:(oc + 1) * 128], zqT[dc],
                            start=(dc == 0), stop=(dc == 7))
                    for sh in range(2):
                        j = 2 * oc + sh
                        for bb in range(B):
                            for hl in range(2):
                                dst = qh[bb, hl].rearrange(
                                    "p (m s) -> p m s", s=16)[0:64, :, j]
                                src = qp[sh * 64:(sh + 1) * 64,
                                         bb * 64 + hl * 32:bb * 64 + hl * 32 + 32]
                                nc.vector.tensor_scalar_add(
                                    out=dst, in0=src,
                                    scalar1=bq_sb[sh * 64:(sh + 1) * 64, oc:oc + 1])

                kh = [apool.tile([65, SKV], bf, tag=f"kh{hl}", name=f"kh{hl}")
                      for hl in range(2)]
                zkv = [apool.tile([128, D], bf, tag=f"zkv{kc}", name=f"zkv{kc}")
                       for kc in range(4)]
                zkvT = [apool.tile([128, 512], bf, tag=f"zkt{dc}", name=f"zkt{dc}")
                        for dc in range(8)]
                rc_t = [apool.tile([128, 1], f32, tag=f"rc{qt}", name=f"rc{qt}")
                        for qt in range(4)]

                for bb in range(B):
                    # ---- Phase A: LN(kv), transpose, K/V projections
                    for kc in range(4):
                        kvt = tmp.tile([128, D], f32, tag="kvt", name="kvt")
                        nc.scalar.dma_start(
                            out=kvt,
                            in_=kvs_d[bb * 512 + kc * 128:
                                      bb * 512 + (kc + 1) * 128, :])
                        _ln(nc, tmp, epst, kvt, zkv[kc])
                    for dc in range(8):
                        for kc in range(4):
                            pt = pst.tile([128, 128], bf, tag="tr", name="tr")
                            nc.tensor.transpose(
                                pt, zkv[kc][:, dc * 128:(dc + 1) * 128], identt)
                            nc.vector.tensor_copy(
                                out=zkvT[dc][:, kc * 128:(kc + 1) * 128],
                                in_=pt)
                    for hl in range(2):
                        nc.sync.dma_start(out=kh[hl][64:65, :],
                                          in_=mask_d[bb:bb + 1, :])
                    for oc in range(8):
                        kpt = pss.tile([128, 512], f32, tag="s", name="s")
                        for dc in range(8):
                            nc.tensor.matmul(
                                kpt, wk_s[dc][:, oc * 128:(oc + 1) * 128],
                                zkvT[dc], start=(dc == 0), stop=(dc == 7))
                        for sh in range(2):
                            j = 2 * oc + sh
                            for hl in range(2):
                                dst = kh[hl].rearrange(
                                    "p (m s) -> p m s", s=16)[0:64, :, j]
                                src = kpt[sh * 64:(sh + 1) * 64,
                                          hl * 256:(hl + 1) * 256]
                                nc.vector.tensor_scalar_add(
                                    out=dst, in0=src,
                                    scalar1=bk_sb[sh * 64:(sh + 1) * 64, oc:oc + 1])
                    vdr = vdp.tile([512 * 1024], bf, tag=f"v{bb}", name=f"v{bb}")
                    vv = vdr.rearrange("(r d) -> r d", d=1024)
                    for rt4 in range(4):
                        for dh in range(2):
                            vpt = pss.tile([128, 512], f32, tag="s", name="s")
                            for dc in range(8):
                                nc.tensor.matmul(
                                    vpt, zkvT[dc][:, rt4 * 128:(rt4 + 1) * 128],
                                    wv_s[dc][:, dh * 512:(dh + 1) * 512],
                                    start=(dc == 0), stop=(dc == 7))
                            vsb = tmp.tile([128, 512], bf, tag="vsb", name="vsb")
                            nc.vector.tensor_tensor(
                                vsb, vpt, bv_b[:, dh * 512:(dh + 1) * 512],
                                ALU.add)
                            nc.gpsimd.dma_start(
                                out=vv[rt4 * 128:(rt4 + 1) * 128,
                                       dh * 512:(dh + 1) * 512],
                                in_=vsb)

                    # ---- Phase B: per head, S, softmax, A-out, A@V
                    for hl in range(2):
                        vhall = vhp.tile([128, 2048], bf, tag="vh", name="vh")
                        nc.scalar.dma_start(
                            out=vhall.rearrange("p (k e) -> p k e", e=64),
                            in_=bass.AP(
                                tensor=vdr.tensor,
                                offset=vdr.offset + hl * 262144,
                                ap=[[64, 128], [8192, 32], [1, 64]]))
                        rs_t = rsp.tile([512], f32, tag="rs", name="rs")
                        for qt in range(4):
                            part = tmp.tile([128, 8], f32, tag="pt", name="pt")
                            ah = [pab.tile([128, 2048], f32, tag="asb",
                                           name=f"asb{i}") for i in range(2)]
                            for kt in range(8):
                                spt = pss.tile([128, 512], f32, tag="s", name="s")
                                nc.tensor.matmul(
                                    spt, qh[bb, hl][:, qt * 128:(qt + 1) * 128],
                                    kh[hl][:, kt * 512:(kt + 1) * 512],
                                    start=True, stop=True)
                                nc.scalar.activation(
                                    out=ah[kt // 4][:, (kt % 4) * 512:
                                                    (kt % 4 + 1) * 512],
                                    in_=spt, func=AF.Exp,
                                    accum_out=part[:, kt:kt + 1])
                            sm = tmp.tile([128, 1], f32, tag="sm", name="sm")
                            nc.vector.tensor_reduce(
                                out=sm, in_=part, axis=mybir.AxisListType.X,
                                op=ALU.add)
                            nc.vector.reciprocal(out=rc_t[qt], in_=sm)
                            nc.sync.dma_start(
                                out=rs_t[qt * 128:(qt + 1) * 128],
                                in_=rc_t[qt])
                            row0 = (bb * 2 + hl) * 512 + qt * 128
                            ring = nc.sync if qt % 2 == 0 else nc.scalar
                            for i in range(2):
                                nc.vector.tensor_scalar_mul(
                                    out=ah[i], in0=ah[i], scalar1=rc_t[qt])
                                ring.dma_start(
                                    out=attn_d[row0:row0 + 128,
                                               i * 2048:(i + 1) * 2048],
                                    in_=ah[i])
                        rband = tmp.tile([64, 512], f32, tag="rb", name="rb")
                        nc.gpsimd.dma_start(out=rband, in_=bass.AP(
                            tensor=rs_t.tensor, offset=rs_t.offset,
                            ap=[[0, 64], [1, 512]]))
                        pavt = psa.tile([128, 512], f32, tag="pav", name="pav")
                        pav = pavt[0:64, :]
                        est_l = {}

                        def emit_st(kt):
                            stp = pss.tile([128, 512], f32, tag="s", name="s")
                            nc.tensor.matmul(
                                stp, kh[hl][:, kt * 128:(kt + 1) * 128],
                                qh[bb, hl], start=True, stop=True)
                            est = tmp.tile([128, 512], bf, tag="est", name="est")
                            nc.scalar.activation(out=est, in_=stp, func=AF.Exp)
                            est_l[kt] = est

                        emit_st(0)
                        for kt in range(32):
                            if kt + 1 < 32:
                                emit_st(kt + 1)
                            nc.tensor.matmul(
                                pav, vhall[:, kt * 64:(kt + 1) * 64],
                                est_l[kt],
                                start=(kt == 0), stop=(kt == 31))
                            del est_l[kt]
                        for j in range(16):
                            pv = pavt.rearrange(
                                "p (m s) -> p m s", s=16)[0:64, :, j]
                            rb = rband.rearrange(
                                "p (m s) -> p m s", s=16)[:, :, j]
                            dc = j // 2
                            prow = 64 * (j % 2)
                            dst = aoT[dc][prow:prow + 64,
                                          bb * 64 + hl * 32:
                                          bb * 64 + hl * 32 + 32]
                            nc.vector.tensor_tensor(dst, pv, rb, ALU.mult)

            # ---- Phase C: O-projection, residual, LN2, MLP
            with tc.tile_pool(name="pc2", bufs=1) as pc2:
                w1_s = []
                for dc in range(8):
                    t = pc2.tile([128, FFW], bf, tag=f"w1{dc}", name=f"w1{dc}")
                    nc.scalar.dma_start(
                        out=t, in_=w1_d[dc * 128:(dc + 1) * 128, :])
                    w1_s.append(t)
                b1_sb = pc2.tile([128, 32], f32, tag="b1", name="b1")
                nc.scalar.dma_start(out=b1_sb, in_=b1_d)
                h1T = [pc2.tile([128, 256], bf, tag=f"h1{fc}", name=f"h1{fc}")
                       for fc in range(32)]

                with tc.tile_pool(name="pc1", bufs=1) as pc1, \
                     tc.tile_pool(name="t2", bufs=3) as t2:
                    wo_s = []
                    for dc in range(8):
                        t = pc1.tile([128, D], bf, tag=f"wo{dc}", name=f"wo{dc}")
                        nc.sync.dma_start(
                            out=t, in_=wo_d[dc * 128:(dc + 1) * 128, :])
                        wo_s.append(t)
                    bo_b = pc1.tile([128, D], f32, tag="bob", name="bob")
                    nc.gpsimd.dma_start(out=bo_b, in_=bass.AP(
                        tensor=bor_d.tensor, offset=bor_d.offset,
                        ap=[[0, 128], [1, D]]))
                    b2_b = pc1.tile([128, D], f32, tag="b2b", name="b2b")
                    nc.gpsimd.dma_start(out=b2_b, in_=bass.AP(
                        tensor=b2r_d.tensor, offset=b2r_d.offset,
                        ap=[[0, 128], [1, D]]))
                    qsb, oat, z2 = [], [], []
                    for rt in range(2):
                        t = pc1.tile([128, D], f32, tag=f"qsb{rt}", name=f"qsb{rt}")
                        nc.vector.tensor_tensor(t, qs_t[rt], bo_b, ALU.add)
                        qsb.append(t)
                        oat.append(pc1.tile([128, D], f32, tag=f"oat{rt}", name=f"oat{rt}"))
                        z2.append(pc1.tile([128, D], bf, tag=f"z2{rt}", name=f"z2{rt}"))
                    for rt in range(2):
                        for dh in range(2):
                            opt = pss.tile([128, 512], f32, tag="s", name="s")
                            for dc in range(8):
                                nc.tensor.matmul(
                                    opt, aoT[dc][:, rt * 128:(rt + 1) * 128],
                                    wo_s[dc][:, dh * 512:(dh + 1) * 512],
                                    start=(dc == 0), stop=(dc == 7))
                            nc.vector.tensor_tensor(
                                oat[rt][:, dh * 512:(dh + 1) * 512], opt,
                                qsb[rt][:, dh * 512:(dh + 1) * 512], ALU.add)
                        nc.vector.tensor_tensor(oat2[rt], oat[rt], b2_b, ALU.add)
                        _ln(nc, t2, epst, oat[rt], z2[rt])
                    for dc in range(8):
                        for rt in range(2):
                            pt = pst.tile([128, 128], bf, tag="tr", name="tr")
                            nc.tensor.transpose(
                                pt, z2[rt][:, dc * 128:(dc + 1) * 128], identt)
                            nc.vector.tensor_copy(
                                out=z2T[dc][:, rt * 128:(rt + 1) * 128], in_=pt)

                # ---- Phase C2: MLP, w2 streamed, MLP1/MLP2 interleaved
                with tc.tile_pool(name="w2p", bufs=6) as w2p, \
                     tc.tile_pool(name="t3", bufs=3) as t3:
                    w2t = {}

                    def load_w2(fc):
                        t = w2p.tile([128, D], bf, tag="w2", name=f"w2s{fc}")
                        nc.sync.dma_start(
                            out=t, in_=w2_d[fc * 128:(fc + 1) * 128, :])
                        w2t[fc] = t

                    fps = [pss.tile([128, 512], f32, tag="s", name=f"mp{i}")
                           for i in range(4)]
                    for fc in range(4):
                        load_w2(fc)

                    def emit_mlp2(fc):
                        wt = w2t.pop(fc)
                        for rt in range(2):
                            for dh in range(2):
                                nc.tensor.matmul(
                                    fps[rt * 2 + dh],
                                    h1T[fc][:, rt * 128:(rt + 1) * 128],
                                    wt[:, dh * 512:(dh + 1) * 512],
                                    start=(fc == 0), stop=(fc == 31))

                    prev = None
                    for fc in range(32):
                        mpt = psa.tile([128, 512], f32, tag="pav", name="pav")
                        mp = mpt[:, 0:256]
                        for dc in range(8):
                            nc.tensor.matmul(
                                mp, w1_s[dc][:, fc * 128:(fc + 1) * 128],
                                z2T[dc], start=(dc == 0), stop=(dc == 7))
                        nc.scalar.activation(out=h1T[fc], in_=mp, func=AF.Gelu,
                                             bias=b1_sb[:, fc:fc + 1])
                        if prev is not None:
                            emit_mlp2(prev)
                        if fc + 4 < 32:
                            load_w2(fc + 4)
                        prev = fc
                    emit_mlp2(31)
                    for rt in range(2):
                        for dh in range(2):
                            ot = t3.tile([128, 512], f32, tag="ot", name="ot")
                            nc.vector.tensor_tensor(
                                ot, fps[rt * 2 + dh],
                                oat2[rt][:, dh * 512:(dh + 1) * 512],
                                ALU.add)
                            nc.sync.dma_start(
                                out=out_d[rt * 128:(rt + 1) * 128,
                                          dh * 512:(dh + 1) * 512],
                                in_=ot)

    nc.compile()
    _cache["nc"] = nc
    return nc


def _prep_inputs(inputs):
    g = {k: np.asarray(v) for k, v in inputs.items()}
    scale = np.float32(D ** -0.5)
    wq_e = ((g["lnq_g"][:, None] * g["Wq"]) * scale).astype(BF16)
    bq_e = ((g["lnq_b"] @ g["Wq"] + g["bq"]) * scale).astype(np.float32)
    wk_e = (g["lnkv_g"][:, None] * g["Wk"]).astype(BF16)
    bk_e = (g["lnkv_b"] @ g["Wk"] + g["bk"]).astype(np.float32)
    wv_e = (g["lnkv_g"][:, None] * g["Wv"]).astype(BF16)
    bv_e = (g["lnkv_b"] @ g["Wv"] + g["bv"]).astype(np.float32)
    w1_e = (g["lnqkv_g"][:, None] * g["W1"]).astype(BF16)
    b1_e = (g["lnqkv_b"] @ g["W1"] + g["b1"]).astype(np.float32)
    common = {
        "maskb": g["attn_mask"].reshape(B, SKV).astype(BF16),
        "wq": wq_e, "wk": wk_e, "wv": wv_e,
        "wo": g["Wo"].astype(BF16),
        "w1": w1_e, "w2": g["W2"].astype(BF16),
        "bq2": np.ascontiguousarray(bq_e.reshape(8, 128).T),
        "bk2": np.ascontiguousarray(bk_e.reshape(8, 128).T),
        "b12": np.ascontiguousarray(b1_e.reshape(32, 128).T),
        "bvr": bv_e.reshape(1, D),
        "bor": g["bo"].astype(np.float32).reshape(1, D),
        "b2r": g["b2"].astype(np.float32).reshape(1, D),
        "ident": np.eye(128, dtype=BF16),
        "epsc": np.full((128, 1), EPS, np.float32),
    }
    q = g["q"].astype(np.float32)
    kv = g["kv"].astype(np.float32)
    in_maps = []
    for c in range(N_CORES):
        m = dict(common)
        m["qs"] = np.ascontiguousarray(
            q[:, 64 * c:64 * (c + 1), :]).reshape(256, D)
        m["kvs"] = np.ascontiguousarray(
            kv[:, 512 * c:512 * (c + 1), :]).reshape(2048, D)
        in_maps.append(m)
    return in_maps


def kernel(**inputs):
    nc = _build()
    in_maps = _prep_inputs(inputs)
    res = run_bass_kernel_spmd(nc, in_maps, core_ids=list(range(N_CORES)))
    out_full = np.empty((B, SQ, D), np.float32)
    A_full = np.empty((B, H, SQ, SKV), np.float32)
    for c in range(N_CORES):
        r = res.results[c]
        out_full[:, 64 * c:64 * (c + 1), :] = r["out"].reshape(B, 64, D)
        A_full[:, 2 * c:2 * c + 2] = r["attn"].reshape(B, 2, SQ, SKV)
    return out_full, A_full

